# Initial kernel scaffold
#
"""Trainium2 Bass kernel for nn_CrossScaleVFE (cross-fiber Gaussian KL sums).

Math: KL[n,m,a,b] separates as 0.5*(<B_m,G_n> - 2 u_n.w_m + d_m + alpha_n) with
per-parent features B = Ob^T Sb^-1 Ob, w = Ob^T Sb^-1 mu_b, d = mu_b^T Sb^-1 mu_b
- K + logdet Sb - 2 log|det Ob|, and per-child features from one solve
Oa [E|u|H] = [P | mu_a | Bbar] (P = Sa + mu mu^T, Bbar = sum_m W[n,m] B_m),
alpha = 2 log|det Oa| - logdet Sa.  sum(W*KL) then needs only per-site dots.

Sharding: fiber grid GA=48 split 8 ways (6 rows per core); each core computes a
partial (belief, model) pair; host sums.  All solves are Gauss-Jordan without
pivoting, batched 128 partitions x slots along the free dim.
"""
import sys
import numpy as np

for p in ('/opt/trn_rl_repo', '/root/.axon_site/_ro/trn_rl_repo'):
    if p not in sys.path:
        sys.path.insert(0, p)

from contextlib import ExitStack

import concourse.bass as bass
import concourse.tile as tile
from concourse import bacc, mybir
from concourse.bass_types import AP
from concourse.bass_utils import run_bass_kernel_spmd

F32 = mybir.dt.float32
ALU = mybir.AluOpType
ACTF = mybir.ActivationFunctionType

N, M, GA, GB, K = 32, 8, 48, 48, 8
GAc = GA // 8            # per-core GA rows
FIB = GAc * GB           # 288 fibers per core
Q = (N * FIB) // 128     # 72 child slots per partition
S = (M * FIB) // 128     # 18 parent slots per partition
WC = 25                  # child augmented row width [Oa | P | mu | Bbar]
WP = 17                  # parent augmented row width [Sb | Ob | mu]
CEXT = Q * K * WC        # child aug per-partition extent (14400)
PEXT = S * K * WP        # parent aug per-partition extent (2448)

_nc_cache = {}


def _ap(t, offset, dims):
    """AP over tensor handle t, offset in elements, dims [(step, num), ...]."""
    h = t.tensor if hasattr(t, 'tensor') else t
    return AP(h, offset, [[st, n] for st, n in dims])


def _gj(nc, aug, w, nslots, ext, scratch, recip_t, pivprod, engine):
    """Full Gauss-Jordan (no pivoting) on aug: per slot an [8, w] block
    [A | R], row-major; after 8 steps R <- A^-1 R.  pivprod accumulates
    the product of pivots (det A)."""
    e = engine
    for k in range(K):
        piv = _ap(aug, k * w + k, [(K * w, nslots)])
        if k == 0:
            nc.scalar.copy(_ap(pivprod, 0, [(1, nslots)]), piv)
        else:
            e.tensor_tensor(_ap(pivprod, 0, [(1, nslots)]),
                            _ap(pivprod, 0, [(1, nslots)]), piv, ALU.mult)
        e.reciprocal(_ap(recip_t, 0, [(1, nslots)]), piv)
        # negf[slot, i] = -A[i,k] / piv   (all 8 rows; row k unused)
        e.scalar_tensor_tensor(
            _ap(scratch, 0, [(K, nslots), (1, K)]),
            _ap(aug, k, [(K * w, nslots), (w, K)]),
            -1.0,
            _ap(recip_t, 0, [(1, nslots), (0, K)]),
            ALU.mult, ALU.mult)
        for i in range(K):
            if i == k:
                continue
            # row_i += negf_i * row_k
            e.tensor_tensor(
                _ap(scratch, K * nslots, [(w, nslots), (1, w)]),
                _ap(scratch, i, [(K, nslots), (0, w)]),
                _ap(aug, k * w, [(K * w, nslots), (1, w)]),
                ALU.mult)
            e.tensor_tensor(
                _ap(aug, i * w, [(K * w, nslots), (1, w)]),
                _ap(aug, i * w, [(K * w, nslots), (1, w)]),
                _ap(scratch, K * nslots, [(w, nslots), (1, w)]),
                ALU.add)
        # scale row k by 1/piv
        e.tensor_tensor(
            _ap(aug, k * w, [(K * w, nslots), (1, w)]),
            _ap(aug, k * w, [(K * w, nslots), (1, w)]),
            _ap(recip_t, 0, [(1, nslots), (0, w)]),
            ALU.mult)


def _gj_logdet(nc, sg, nslots, scratch, recip_t, pivprod, engine):
    """Forward-only no-pivot elimination on [8,8] blocks (slot ext 64);
    only the pivot product is kept (for logdet of SPD matrices)."""
    e = engine
    for k in range(K):
        piv = _ap(sg, k * 8 + k, [(64, nslots)])
        if k == 0:
            nc.scalar.copy(_ap(pivprod, 0, [(1, nslots)]), piv)
        else:
            e.tensor_tensor(_ap(pivprod, 0, [(1, nslots)]),
                            _ap(pivprod, 0, [(1, nslots)]), piv, ALU.mult)
        if k == K - 1:
            break
        e.reciprocal(_ap(recip_t, 0, [(1, nslots)]), piv)
        nrem = K - 1 - k
        e.scalar_tensor_tensor(
            _ap(scratch, 0, [(1, nslots), (nslots, nrem)]),
            _ap(sg, (k + 1) * 8 + k, [(64, nslots), (8, nrem)]),
            -1.0,
            _ap(recip_t, 0, [(1, nslots), (0, nrem)]),
            ALU.mult, ALU.mult)
        wrem = K - k
        for i in range(k + 1, K):
            e.tensor_tensor(
                _ap(scratch, K * nslots, [(wrem, nslots), (1, wrem)]),
                _ap(scratch, (i - k - 1) * nslots, [(1, nslots), (0, wrem)]),
                _ap(sg, k * 8 + k, [(64, nslots), (1, wrem)]),
                ALU.mult)
            e.tensor_tensor(
                _ap(sg, i * 8 + k, [(64, nslots), (1, wrem)]),
                _ap(sg, i * 8 + k, [(64, nslots), (1, wrem)]),
                _ap(scratch, K * nslots, [(wrem, nslots), (1, wrem)]),
                ALU.add)


def build_bass():
    nc = bacc.Bacc(None, target_bir_lowering=False)
    dr = {}
    cshape = [N, GAc, GB, K, K]
    pshape = [M, GAc, GB, K, K]
    for nm in ('sigma_p', 'omega_c', 'sigma_r', 'omega_model_c'):
        dr[nm] = nc.declare_dram_parameter(nm, cshape, F32)
    for nm in ('mu_p', 'mu_r'):
        dr[nm] = nc.declare_dram_parameter(nm, [N, GAc, GB, K], F32)
    for nm in ('sigma_q', 'omega_par', 'sigma_s', 'omega_model_p'):
        dr[nm] = nc.declare_dram_parameter(nm, pshape, F32)
    for nm in ('mu_q', 'mu_s'):
        dr[nm] = nc.declare_dram_parameter(nm, [M, GAc, GB, K], F32)
    dr['W'] = nc.declare_dram_parameter('W', [N, M, GAc, GB], F32)
    out = nc.declare_dram_parameter('out', [128, 2], F32, isOutput=True)

    kls = [  # (mu_a, sigma_a, omega_a, mu_b, sigma_b, omega_b)
        ('mu_p', 'sigma_p', 'omega_c', 'mu_q', 'sigma_q', 'omega_par'),
        ('mu_r', 'sigma_r', 'omega_model_c', 'mu_s', 'sigma_s', 'omega_model_p'),
    ]

    with ExitStack() as ctx, tile.TileContext(nc) as tc:
        pool = ctx.enter_context(tc.tile_pool(name="main", bufs=1))
        ppool = ctx.enter_context(tc.tile_pool(name="psum", bufs=2, space="PSUM"))

        # W for TensorE: [8*fh + m partitions, n*S + fl]
        wt = pool.tile([128, N * S], F32, name="wt")
        nc.sync.dma_start(
            out=_ap(wt, 0, [(8 * N * S, 16), (N * S, 8), (S, N), (1, S)]),
            in_=_ap(dr['W'], 0, [(S, 16), (M * FIB // M, 8), (M * FIB, N), (1, S)]))
        # NOTE: W strides (elements): n: M*FIB=2304, m: FIB=288, fib: 1
        # dims iterate (fh, m, n, fl): src steps (S, FIB, 2304, 1)

        outsb = pool.tile([128, 2], F32, name="outsb")

        for kl, (nm_mu_a, nm_sg_a, nm_om_a, nm_mu_b, nm_sg_b, nm_om_b) in enumerate(kls):
            tag = f"kl{kl}"
            # ---------------- parent phase ----------------
            augP = pool.tile([128, PEXT], F32, name=f"augP{kl}", tag="augP")
            omB = pool.tile([128, S * 64], F32, name=f"omB{kl}", tag="omB")
            muB = pool.tile([128, S * K], F32, name=f"muB{kl}", tag="muB")
            feat = pool.tile([128, S * 74], F32, name=f"feat{kl}", tag="feat")
            scr_p = pool.tile([128, max(S * WP, S * K) + K * S], F32,
                              name=f"scrp{kl}", tag="scrp")
            recip_p = pool.tile([128, S], F32, name=f"recp{kl}", tag="recp")
            pp_sb = pool.tile([128, S], F32, name=f"ppsb{kl}", tag="ppsb")
            pp_ob = pool.tile([128, S], F32, name=f"ppob{kl}", tag="ppob")

            # DMA parent sigma directly into aug cols 0..7 is 5-dim; stage via scratch
            sgB = pool.tile([128, S * 64], F32, name=f"sgB{kl}", tag="sgB")
            for t, nm in ((sgB, nm_sg_b), (omB, nm_om_b)):
                nc.sync.dma_start(
                    out=_ap(t, 0, [(8 * S * 64, 16), (S * 64, 8), (64, S), (1, 64)]),
                    in_=_ap(dr[nm], 0, [(S * 64, 16), (FIB * 64, 8), (64, S), (1, 64)]))
            nc.sync.dma_start(
                out=_ap(muB, 0, [(8 * S * K, 16), (S * K, 8), (K, S), (1, K)]),
                in_=_ap(dr[nm_mu_b], 0, [(S * K, 16), (FIB * K, 8), (K, S), (1, K)]))

            # assemble parent augmented [Sb | Ob | mu]
            nc.scalar.copy(_ap(augP, 0, [(K * WP, S), (WP, K), (1, K)]),
                           _ap(sgB, 0, [(64, S), (8, K), (1, K)]))
            nc.scalar.copy(_ap(augP, 8, [(K * WP, S), (WP, K), (1, K)]),
                           _ap(omB, 0, [(64, S), (8, K), (1, K)]))
            nc.scalar.copy(_ap(augP, 16, [(K * WP, S), (WP, K)]),
                           _ap(muB, 0, [(K, S), (1, K)]))

            _gj(nc, augP, WP, S, PEXT, scr_p, recip_p, pp_sb, nc.vector)
            _gj_logdet(nc, omB, S, scr_p, recip_p, pp_ob, nc.vector)

            # features: B_ij = sum_k Ob[k,i] * X[k,j];  X = aug cols 8..15
            # reuse omB? omB was destroyed by _gj_logdet! stage a second copy
            # -> instead recompute from DMA: cheaper: copy omB before logdet
            # (omB2 allocated below, copied before _gj_logdet would be better;
            #  simplest: use augP cols 8..15 original? destroyed too. Use omB2.)
            for i in range(K):
                for j in range(K):
                    nc.vector.tensor_tensor(
                        _ap(scr_p, 0, [(8, S), (1, K)]),
                        _ap(omB2, i, [(64, S), (8, K)]),
                        _ap(augP, 8 + j, [(K * WP, S), (WP, K)]),
                        ALU.mult)
                    nc.vector.tensor_reduce(
                        _ap(feat, i * 8 + j, [(74, S)]),
                        _ap(scr_p, 0, [(8, S), (1, K)]),
                        mybir.AxisListType.X, ALU.add)
            for i in range(K):
                nc.vector.tensor_tensor(
                    _ap(scr_p, 0, [(8, S), (1, K)]),
                    _ap(omB2, i, [(64, S), (8, K)]),
                    _ap(augP, 16, [(K * WP, S), (WP, K), (0, 1)]),
                    ALU.mult)
                nc.vector.tensor_reduce(
                    _ap(feat, 64 + i, [(74, S)]),
                    _ap(scr_p, 0, [(8, S), (1, K)]),
                    mybir.AxisListType.X, ALU.add)
            # c = mu . X_mu
            nc.vector.tensor_tensor(
                _ap(scr_p, 0, [(8, S), (1, K)]),
                _ap(muB, 0, [(K, S), (1, K)]),
                _ap(augP, 16, [(K * WP, S), (WP, K)]),
                ALU.mult)
            nc.vector.tensor_reduce(
                _ap(scr_p, 8 * S, [(1, S)]),
                _ap(scr_p, 0, [(8, S), (1, K)]),
                mybir.AxisListType.X, ALU.add)
            # d = c - K + ln(pp_sb) - ln(pp_ob^2)
            nc.scalar.activation(_ap(scr_p, 9 * S, [(1, S)]),
                                 _ap(pp_sb, 0, [(1, S)]), ACTF.Ln)
            nc.scalar.activation(_ap(scr_p, 10 * S, [(1, S)]),
                                 _ap(pp_ob, 0, [(1, S)]), ACTF.Square)
            nc.scalar.activation(_ap(scr_p, 11 * S, [(1, S)]),
                                 _ap(scr_p, 10 * S, [(1, S)]), ACTF.Ln)
            nc.vector.tensor_tensor(_ap(scr_p, 8 * S, [(1, S)]),
                                    _ap(scr_p, 8 * S, [(1, S)]),
                                    _ap(scr_p, 9 * S, [(1, S)]), ALU.add)
            nc.vector.scalar_tensor_tensor(
                _ap(feat, 72, [(74, S)]),
                _ap(scr_p, 11 * S, [(1, S)]), -1.0,
                _ap(scr_p, 8 * S, [(1, S)]), ALU.mult, ALU.add)
            nc.vector.tensor_scalar_add(_ap(feat, 72, [(74, S)]),
                                        _ap(feat, 72, [(74, S)]), -float(K))
            nc.vector.memset(_ap(feat, 73, [(74, S)]), 1.0)

            # ---------------- W contraction on TensorE ----------------
            augC = pool.tile([128, CEXT], F32, name=f"augC{kl}", tag="augC")
            aux = pool.tile([128, Q * 10], F32, name=f"aux{kl}", tag="aux")
            for r in range(4):
                for qg in range(Q // 6):
                    ps = ppool.tile([32, 6 * 74], F32, name=f"ps{kl}_{r}_{qg}",
                                    tag="ps")
                    for g in range(6):
                        q = qg * 6 + g
                        fib = q * 4 + r
                        fh, fl = divmod(fib, S)
                        nc.tensor.matmul(
                            ctx,
                            ps[:, g * 74:(g + 1) * 74],
                            _ap(wt, (8 * fh) * (N * S) + fl, [(N * S, 8), (S, N)]),
                            _ap(feat, (8 * fh) * (S * 74) + fl * 74,
                                [(S * 74, 8), (1, 74)]),
                            start=True, stop=True)
                    q0 = qg * 6
                    nc.scalar.copy(
                        _ap(augC, (32 * r) * CEXT + q0 * (K * WC) + 17,
                            [(CEXT, 32), (K * WC, 6), (WC, K), (1, K)]),
                        _ap(ps, 0, [(6 * 74, 32), (74, 6), (8, K), (1, K)]))
                    nc.scalar.copy(
                        _ap(aux, (32 * r) * (Q * 10) + q0 * 10,
                            [(Q * 10, 32), (10, 6), (1, 10)]),
                        _ap(ps, 64, [(6 * 74, 32), (74, 6), (1, 10)]))

            # ---------------- child phase ----------------
            sgA = pool.tile([128, Q * 64], F32, name=f"sgA{kl}", tag="sgA")
            omA = pool.tile([128, Q * 64], F32, name=f"omA{kl}", tag="omA")
            muA = pool.tile([128, Q * K], F32, name=f"muA{kl}", tag="muA")
            scr_c = pool.tile([128, Q * WC + K * Q], F32, name=f"scrc{kl}",
                              tag="scrc")
            recip_c = pool.tile([128, Q], F32, name=f"recc{kl}", tag="recc")
            pp_oa = pool.tile([128, Q], F32, name=f"ppoa{kl}", tag="ppoa")
            pp_sa = pool.tile([128, Q], F32, name=f"ppsa{kl}", tag="ppsa")

            for t, nm in ((sgA, nm_sg_a), (omA, nm_om_a)):
                nc.sync.dma_start(
                    out=_ap(t, 0, [(32 * Q * 64, 4), (Q * 64, 32), (64, Q), (1, 64)]),
                    in_=_ap(dr[nm], 0, [(64, 4), (FIB * 64, 32), (256, Q), (1, 64)]))
            nc.sync.dma_start(
                out=_ap(muA, 0, [(32 * Q * K, 4), (Q * K, 32), (K, Q), (1, K)]),
                in_=_ap(dr[nm_mu_a], 0, [(K, 4), (FIB * K, 32), (4 * K, Q), (1, K)]))

            # aug cols 0..7 = Oa ; col 16 = mu ; cols 8..15 = P = Sa + mu mu^T
            nc.scalar.copy(_ap(augC, 0, [(K * WC, Q), (WC, K), (1, K)]),
                           _ap(omA, 0, [(64, Q), (8, K), (1, K)]))
            nc.scalar.copy(_ap(augC, 16, [(K * WC, Q), (WC, K)]),
                           _ap(muA, 0, [(K, Q), (1, K)]))
            for i in range(K):
                nc.vector.tensor_tensor(
                    _ap(scr_c, 0, [(K, Q), (1, K)]),
                    _ap(muA, 0, [(K, Q), (1, K)]),
                    _ap(muA, i, [(K, Q), (0, K)]),
                    ALU.mult)
                nc.vector.tensor_tensor(
                    _ap(augC, i * WC + 8, [(K * WC, Q), (1, K)]),
                    _ap(scr_c, 0, [(K, Q), (1, K)]),
                    _ap(sgA, i * 8, [(64, Q), (1, K)]),
                    ALU.add)

            _gj(nc, augC, WC, Q, CEXT, scr_c, recip_c, pp_oa, nc.vector)
            _gj_logdet(nc, sgA, Q, scr_c, recip_c, pp_sa, nc.vector)

            # tr = <H, E>
            nc.vector.tensor_tensor(
                _ap(scr_c, 0, [(64, Q), (8, K), (1, K)]),
                _ap(augC, 8, [(K * WC, Q), (WC, K), (1, K)]),
                _ap(augC, 17, [(K * WC, Q), (WC, K), (1, K)]),
                ALU.mult)
            nc.vector.tensor_reduce(
                _ap(scr_c, Q * 64, [(1, Q)]),
                _ap(scr_c, 0, [(64, Q), (8, K), (1, K)]),
                mybir.AxisListType.XY, ALU.add)
            # cross = u . wbar
            nc.vector.tensor_tensor(
                _ap(scr_c, Q * 65, [(K, Q), (1, K)]),
                _ap(augC, 16, [(K * WC, Q), (WC, K)]),
                _ap(aux, 0, [(10, Q), (1, K)]),
                ALU.mult)
            nc.vector.tensor_reduce(
                _ap(scr_c, Q * 73, [(1, Q)]),
                _ap(scr_c, Q * 65, [(K, Q), (1, K)]),
                mybir.AxisListType.X, ALU.add)
            # alpha = ln(pp_oa^2) - ln(pp_sa)
            nc.scalar.activation(_ap(scr_c, Q * 74, [(1, Q)]),
                                 _ap(pp_oa, 0, [(1, Q)]), ACTF.Square)
            nc.scalar.activation(_ap(scr_c, Q * 74, [(1, Q)]),
                                 _ap(scr_c, Q * 74, [(1, Q)]), ACTF.Ln)
            nc.scalar.activation(_ap(scr_c, Q * 75, [(1, Q)]),
                                 _ap(pp_sa, 0, [(1, Q)]), ACTF.Ln)
            nc.vector.tensor_tensor(_ap(scr_c, Q * 74, [(1, Q)]),
                                    _ap(scr_c, Q * 74, [(1, Q)]),
                                    _ap(scr_c, Q * 75, [(1, Q)]), ALU.subtract)
            # contrib = tr - 2*cross + sbar + alpha*Wbar   (0.5 factor at end)
            nc.vector.scalar_tensor_tensor(
                _ap(scr_c, Q * 76, [(1, Q)]),
                _ap(scr_c, Q * 73, [(1, Q)]), -2.0,
                _ap(scr_c, Q * 64, [(1, Q)]), ALU.mult, ALU.add)
            nc.vector.tensor_tensor(
                _ap(scr_c, Q * 76, [(1, Q)]),
                _ap(scr_c, Q * 76, [(1, Q)]),
                _ap(aux, 8, [(10, Q)]), ALU.add)
            nc.vector.tensor_tensor(
                _ap(scr_c, Q * 77, [(1, Q)]),
                _ap(scr_c, Q * 74, [(1, Q)]),
                _ap(aux, 9, [(10, Q)]), ALU.mult)
            nc.vector.tensor_tensor(
                _ap(scr_c, Q * 76, [(1, Q)]),
                _ap(scr_c, Q * 76, [(1, Q)]),
                _ap(scr_c, Q * 77, [(1, Q)]), ALU.add)
            nc.vector.tensor_reduce(
                _ap(scr_c, Q * 78, [(1, 1)]),
                _ap(scr_c, Q * 76, [(1, Q)]),
                mybir.AxisListType.X, ALU.add)
            nc.scalar.activation(_ap(outsb, kl, [(2, 1)]),
                                 _ap(scr_c, Q * 78, [(1, 1)]),
                                 ACTF.Copy, scale=0.5)

        nc.sync.dma_start(out=out[:, :], in_=outsb[:, :])
    return nc


def _build():
    if 'nc' not in _nc_cache:
        _nc_cache['nc'] = build_bass()
    return _nc_cache['nc']


def kernel(**inputs):
    nc = _build()
    ins = {k: np.ascontiguousarray(np.asarray(v, dtype=np.float32))
           for k, v in inputs.items()}
    in_maps = []
    for c in range(8):
        sl = slice(c * GAc, (c + 1) * GAc)
        m = {}
        for nm in ('mu_p', 'sigma_p', 'omega_c', 'mu_q', 'sigma_q', 'omega_par',
                   'mu_r', 'sigma_r', 'omega_model_c', 'mu_s', 'sigma_s',
                   'omega_model_p'):
            m[nm] = np.ascontiguousarray(ins[nm][:, sl])
        m['W'] = np.ascontiguousarray(ins['W'][:, :, sl])
        in_maps.append(m)
    res = run_bass_kernel_spmd(nc, in_maps, core_ids=list(range(8)))
    parts = np.stack([r['out'] for r in res.results])   # [8, 128, 2]
    belief = np.float32(parts[:, :, 0].astype(np.float64).sum())
    model = np.float32(parts[:, :, 1].astype(np.float64).sum())
    total = np.float32(np.float64(belief) + np.float64(model))
    return total, belief, model


if __name__ == '__main__':
    nc = build_bass()
    print("built OK")


# revision 21
# speedup vs baseline: 24.0372x; 24.0372x over previous
"""Trainium2 Bass kernel for nn_CrossScaleVFE (cross-fiber Gaussian KL sums).

KL[n,m,a,b] = 0.5*(<B_m,G_n> - 2 u_n.w_m + d_m + alpha_n):
  parent: B = Ob^T Sb^-1 Ob, w = Ob^T Sb^-1 mu_b,
          d = mu_b^T Sb^-1 mu_b - K + logdet Sb - 2 log|det Ob|
  child:  solve Oa [E|u|H] = [P | mu_a | Bbar], P = Sa + mu mu^T,
          Bbar/wbar/sbar/Wbar = sum_m W[n,m]*(B_m, w_m, d_m, 1)  (TensorE)
  site contribution = 0.5*(<H,E> - 2 u.wbar + sbar + alpha*Wbar)

Sharding: fiber grid GA=48 split 8 ways (6 per core); host sums 8 partial pairs.
Solves: batched no-pivot Gauss-Jordan, 128 partitions x slots along free dim.
"""
import sys
import numpy as np

for p in ('/opt/trn_rl_repo', '/root/.axon_site/_ro/trn_rl_repo'):
    if p not in sys.path:
        sys.path.insert(0, p)

from contextlib import ExitStack

import concourse.bass as bass
import concourse.tile as tile
from concourse import bacc, mybir
from concourse.bass_types import AP
from concourse.bass_utils import run_bass_kernel_spmd

F32 = mybir.dt.float32
ALU = mybir.AluOpType
ACTF = mybir.ActivationFunctionType

N, M, GA, GB, K = 32, 8, 48, 48, 8
GAc = GA // 8
FIB = GAc * GB           # 288 fibers / core
Q = (N * FIB) // 128     # 72 child slots / partition
S = (M * FIB) // 128     # 18 parent slots / partition
WC = 25                  # child aug row width [Oa | P | mu | Bbar]
WP = 17                  # parent aug row width [Sb | Ob | mu]

_nc_cache = {}


def _mk(t, ext, off, dims, nparts=128, pstep=None):
    h = t.tensor if hasattr(t, 'tensor') else t
    pd = [[ext if pstep is None else pstep, nparts]]
    return AP(h, off, pd + [[st, n] for st, n in dims])


def _gj(nc, aug, ext, w, ns, scr, sext, recip, rext, pp, e):
    """No-pivot Gauss-Jordan on per-slot [8, w] row-major blocks [A | R];
    R <- A^-1 R, pp <- prod of pivots.  scr layout: negf [ns*K], tmp [ns*w]."""
    for k in range(K):
        piv = _mk(aug, ext, k * w + k, [(K * w, ns)])
        ppap = _mk(pp, rext, 0, [(1, ns)])
        if k == 0:
            nc.scalar.copy(ppap, piv)
        else:
            e.tensor_tensor(ppap, ppap, piv, ALU.mult)
        pool_mode = not hasattr(e, 'reciprocal')
        nc.vector.reciprocal(_mk(recip, rext, 0, [(1, ns)]), piv)
        if pool_mode:
            nc.vector.tensor_scalar_mul(_mk(recip, rext, ns, [(1, ns)]),
                                        _mk(recip, rext, 0, [(1, ns)]), -1.0)
            e.tensor_tensor(
                _mk(scr, sext, 0, [(K, ns), (1, K)]),
                _mk(aug, ext, k, [(K * w, ns), (w, K)]),
                _mk(recip, rext, ns, [(1, ns), (0, K)]),
                ALU.mult)
        else:
            e.scalar_tensor_tensor(
                _mk(scr, sext, 0, [(K, ns), (1, K)]),
                _mk(aug, ext, k, [(K * w, ns), (w, K)]),
                -1.0,
                _mk(recip, rext, 0, [(1, ns), (0, K)]),
                ALU.mult, ALU.mult)
        for i in range(K):
            if i == k:
                continue
            e.tensor_tensor(
                _mk(scr, sext, K * ns, [(w, ns), (1, w)]),
                _mk(scr, sext, i, [(K, ns), (0, w)]),
                _mk(aug, ext, k * w, [(K * w, ns), (1, w)]),
                ALU.mult)
            e.tensor_tensor(
                _mk(aug, ext, i * w, [(K * w, ns), (1, w)]),
                _mk(aug, ext, i * w, [(K * w, ns), (1, w)]),
                _mk(scr, sext, K * ns, [(w, ns), (1, w)]),
                ALU.add)
        e.tensor_tensor(
            _mk(aug, ext, k * w, [(K * w, ns), (1, w)]),
            _mk(aug, ext, k * w, [(K * w, ns), (1, w)]),
            _mk(recip, rext, 0, [(1, ns), (0, w)]),
            ALU.mult)


def _gj_logdet(nc, sg, ext, ns, scr, sext, recip, rext, pp, e):
    """Forward elimination on per-slot [8,8] blocks; keeps only pivot product."""
    for k in range(K):
        piv = _mk(sg, ext, k * 8 + k, [(64, ns)])
        ppap = _mk(pp, rext, 0, [(1, ns)])
        if k == 0:
            nc.scalar.copy(ppap, piv)
        else:
            e.tensor_tensor(ppap, ppap, piv, ALU.mult)
        if k == K - 1:
            break
        pool_mode = not hasattr(e, 'reciprocal')
        nc.vector.reciprocal(_mk(recip, rext, 0, [(1, ns)]), piv)
        nrem = K - 1 - k
        wrem = K - k
        if pool_mode:
            nc.vector.tensor_scalar_mul(_mk(recip, rext, ns, [(1, ns)]),
                                        _mk(recip, rext, 0, [(1, ns)]), -1.0)
            e.tensor_tensor(
                _mk(scr, sext, 0, [(1, ns), (ns, nrem)]),
                _mk(sg, ext, (k + 1) * 8 + k, [(64, ns), (8, nrem)]),
                _mk(recip, rext, ns, [(1, ns), (0, nrem)]),
                ALU.mult)
        else:
            e.scalar_tensor_tensor(
                _mk(scr, sext, 0, [(1, ns), (ns, nrem)]),
                _mk(sg, ext, (k + 1) * 8 + k, [(64, ns), (8, nrem)]),
                -1.0,
                _mk(recip, rext, 0, [(1, ns), (0, nrem)]),
                ALU.mult, ALU.mult)
        for i in range(k + 1, K):
            e.tensor_tensor(
                _mk(scr, sext, K * ns, [(wrem, ns), (1, wrem)]),
                _mk(scr, sext, (i - k - 1) * ns, [(1, ns), (0, wrem)]),
                _mk(sg, ext, k * 8 + k, [(64, ns), (1, wrem)]),
                ALU.mult)
            e.tensor_tensor(
                _mk(sg, ext, i * 8 + k, [(64, ns), (1, wrem)]),
                _mk(sg, ext, i * 8 + k, [(64, ns), (1, wrem)]),
                _mk(scr, sext, K * ns, [(wrem, ns), (1, wrem)]),
                ALU.add)


def build_bass():
    nc = bacc.Bacc(None, target_bir_lowering=False)
    dr = {}
    for nm in ('sigma_p', 'omega_c', 'sigma_r', 'omega_model_c'):
        dr[nm] = nc.declare_dram_parameter(nm, [N, GAc, GB, K, K], F32, isOutput=False)
    for nm in ('mu_p', 'mu_r'):
        dr[nm] = nc.declare_dram_parameter(nm, [N, GAc, GB, K], F32, isOutput=False)
    for nm in ('sigma_q', 'omega_par', 'sigma_s', 'omega_model_p'):
        dr[nm] = nc.declare_dram_parameter(nm, [M, GAc, GB, K, K], F32, isOutput=False)
    for nm in ('mu_q', 'mu_s'):
        dr[nm] = nc.declare_dram_parameter(nm, [M, GAc, GB, K], F32, isOutput=False)
    dr['W'] = nc.declare_dram_parameter("W", [N, M, GAc, GB], F32, isOutput=False)
    out = nc.declare_dram_parameter('out', [128, 2], F32, isOutput=True)

    kls = [
        ('mu_p', 'sigma_p', 'omega_c', 'mu_q', 'sigma_q', 'omega_par'),
        ('mu_r', 'sigma_r', 'omega_model_c', 'mu_s', 'sigma_s', 'omega_model_p'),
    ]

    CE = Q * K * WC          # child aug extent / partition (14400)
    PE = S * K * WP          # parent aug extent (2448)
    SCE = Q * 80             # child scratch extent
    SPE = 512                # parent scratch extent
    WTE = N * S              # W-tile extent (576)
    FEE = S * 74             # feature extent

    with tile.TileContext(nc) as tc, \
            tc.tile_pool(name="main", bufs=1) as pool, \
            tc.tile_pool(name="psum", bufs=2, space="PSUM") as ppool:

        # W for TensorE: fib = 96*b + wl ; Wt2[32b + m, n*96 + wl]
        WT2 = N * 96
        wt = pool.tile([128, WT2], F32, name="wt")
        for m in range(M):
            nc.sync.dma_start(
                out=_mk(wt, WT2, m * WT2, [(96, N), (1, 96)],
                        nparts=3, pstep=32 * WT2),
                in_=_mk(dr['W'], 0, m * FIB, [(M * FIB, N), (1, 96)],
                        nparts=3, pstep=96))
        outsb = pool.tile([128, 2], F32, name="outsb")

        for kl, (nmua, nmsa, nmoa, nmub, nmsb, nmob) in enumerate(kls):
            # ---------------- parent phase ----------------
            augP = pool.tile([128, PE], F32, name=f"augP{kl}", tag="augP")
            sgB = pool.tile([128, S * 64], F32, name=f"sgB{kl}", tag="sgB")
            omB = pool.tile([128, S * 64], F32, name=f"omB{kl}", tag="omB")
            omB2 = pool.tile([128, S * 64], F32, name=f"omB2{kl}", tag="omB2")
            muB = pool.tile([128, S * K], F32, name=f"muB{kl}", tag="muB")
            feat = pool.tile([128, FEE], F32, name=f"feat{kl}", tag="feat")
            scp = pool.tile([128, SPE], F32, name=f"scp{kl}", tag="scp")
            rcp = pool.tile([128, 2 * S], F32, name=f"rcp{kl}", tag="rcp")
            ppsb = pool.tile([128, 2 * S], F32, name=f"ppsb{kl}", tag="ppsb")
            ppob = pool.tile([128, 2 * S], F32, name=f"ppob{kl}", tag="ppob")

            # parent site layout: fib = 96b + 16h + r -> partition 8r+m, slot 6b+h
            # per-m DMAs, dims (r, bh, e): partition crossing only in dim0
            for m in range(M):
                for t, nm in ((sgB, nmsb), (omB, nmob)):
                    nc.sync.dma_start(
                        out=_mk(t, 0, m * S * 64, [(64, S), (1, 64)],
                                nparts=16, pstep=8 * S * 64),
                        in_=_mk(dr[nm], 0, m * FIB * 64, [(16 * 64, S), (1, 64)],
                                nparts=16, pstep=64))
                nc.sync.dma_start(
                    out=_mk(muB, 0, m * S * K, [(K, S), (1, K)],
                            nparts=16, pstep=8 * S * K),
                    in_=_mk(dr[nmub], 0, m * FIB * K, [(16 * K, S), (1, K)],
                            nparts=16, pstep=K))

            nc.scalar.copy(_mk(augP, PE, 0, [(K * WP, S), (WP, K), (1, K)]),
                           _mk(sgB, S * 64, 0, [(64, S), (8, K), (1, K)]))
            nc.scalar.copy(_mk(augP, PE, 8, [(K * WP, S), (WP, K), (1, K)]),
                           _mk(omB, S * 64, 0, [(64, S), (8, K), (1, K)]))
            nc.scalar.copy(_mk(augP, PE, 16, [(K * WP, S), (WP, K)]),
                           _mk(muB, S * K, 0, [(K, S), (1, K)]))
            nc.scalar.copy(_mk(omB2, S * 64, 0, [(1, S * 64)]),
                           _mk(omB, S * 64, 0, [(1, S * 64)]))

            _gj(nc, augP, PE, WP, S, scp, SPE, rcp, 2 * S, ppsb, nc.gpsimd)
            _gj_logdet(nc, omB, S * 64, S, scp, SPE, rcp, 2 * S, ppob,
                       nc.gpsimd)

            # B_ij = sum_k Ob[k,i] X[k,j], X = Sb^-1 Ob at aug cols 8..15
            for i in range(K):
                for j in range(K):
                    nc.vector.tensor_tensor(
                        _mk(scp, SPE, 0, [(8, S), (1, K)]),
                        _mk(omB2, S * 64, i, [(64, S), (8, K)]),
                        _mk(augP, PE, 8 + j, [(K * WP, S), (WP, K)]),
                        ALU.mult)
                    nc.vector.tensor_reduce(
                        _mk(feat, FEE, i * 8 + j, [(74, S)]),
                        _mk(scp, SPE, 0, [(8, S), (1, K)]),
                        mybir.AxisListType.X, ALU.add)
            for i in range(K):
                nc.vector.tensor_tensor(
                    _mk(scp, SPE, 0, [(8, S), (1, K)]),
                    _mk(omB2, S * 64, i, [(64, S), (8, K)]),
                    _mk(augP, PE, 16, [(K * WP, S), (WP, K)]),
                    ALU.mult)
                nc.vector.tensor_reduce(
                    _mk(feat, FEE, 64 + i, [(74, S)]),
                    _mk(scp, SPE, 0, [(8, S), (1, K)]),
                    mybir.AxisListType.X, ALU.add)
            nc.vector.tensor_tensor(
                _mk(scp, SPE, 0, [(8, S), (1, K)]),
                _mk(muB, S * K, 0, [(K, S), (1, K)]),
                _mk(augP, PE, 16, [(K * WP, S), (WP, K)]),
                ALU.mult)
            nc.vector.tensor_reduce(
                _mk(scp, SPE, 8 * S, [(1, S)]),
                _mk(scp, SPE, 0, [(8, S), (1, K)]),
                mybir.AxisListType.X, ALU.add)
            nc.scalar.activation(_mk(scp, SPE, 9 * S, [(1, S)]),
                                 _mk(ppsb, 2 * S, 0, [(1, S)]), ACTF.Ln)
            nc.scalar.activation(_mk(scp, SPE, 10 * S, [(1, S)]),
                                 _mk(ppob, 2 * S, 0, [(1, S)]), ACTF.Square)
            nc.scalar.activation(_mk(scp, SPE, 11 * S, [(1, S)]),
                                 _mk(scp, SPE, 10 * S, [(1, S)]), ACTF.Ln)
            nc.vector.tensor_tensor(_mk(scp, SPE, 8 * S, [(1, S)]),
                                    _mk(scp, SPE, 8 * S, [(1, S)]),
                                    _mk(scp, SPE, 9 * S, [(1, S)]), ALU.add)
            nc.vector.scalar_tensor_tensor(
                _mk(feat, FEE, 72, [(74, S)]),
                _mk(scp, SPE, 11 * S, [(1, S)]), -1.0,
                _mk(scp, SPE, 8 * S, [(1, S)]), ALU.mult, ALU.add)
            nc.vector.tensor_scalar_add(_mk(feat, FEE, 72, [(74, S)]),
                                        _mk(feat, FEE, 72, [(74, S)]), -float(K))
            nc.vector.memset(_mk(feat, FEE, 73, [(74, S)]), 1.0)

            # re-layout features for PE quadrant bases: featM[32b+m, wl*74+e]
            FME = 96 * 74
            featM = pool.tile([128, FME], F32, name=f"featM{kl}", tag="featM")
            # per (b, r): dims (m, h, e) — partition crossing only via m in dim0
            for b in range(3):
                for r in range(16):
                    nc.sync.dma_start(
                        out=_mk(featM, 0, 32 * b * FME + (16 * 0 + r) * 74,
                                [(16 * 74, 6), (1, 74)], nparts=8, pstep=FME),
                        in_=_mk(feat, 0, 8 * r * FEE + b * 6 * 74,
                                [(74, 6), (1, 74)], nparts=8, pstep=FEE))

            # ---------------- W contraction (TensorE) ----------------
            augC = pool.tile([128, CE], F32, name=f"augC{kl}", tag="augC")
            aux = pool.tile([128, Q * 10], F32, name=f"aux{kl}", tag="aux")
            for r in range(4):
                for qg in range(Q // 6):
                    ps = ppool.tile([32, 6 * 74], F32, name=f"ps{kl}_{r}_{qg}",
                                    tag="ps")
                    for g in range(6):
                        q = qg * 6 + g
                        fib = q * 4 + r
                        b, wl = divmod(fib, 96)
                        nc.tensor.matmul(
                            ps[:, g * 74:(g + 1) * 74],
                            _mk(wt, WT2, 32 * b * WT2 + wl, [(96, N)], nparts=8),
                            _mk(featM, FME, 32 * b * FME + wl * 74, [(1, 74)],
                                nparts=8),
                            start=True, stop=True)
                    q0 = qg * 6
                    nc.scalar.copy(
                        _mk(augC, CE, 32 * r * CE + q0 * (K * WC) + 17,
                            [(K * WC, 6), (WC, K), (1, K)], nparts=32),
                        _mk(ps, 6 * 74, 0, [(74, 6), (8, K), (1, K)], nparts=32))
                    nc.scalar.copy(
                        _mk(aux, Q * 10, 32 * r * (Q * 10) + q0 * 10,
                            [(10, 6), (1, 10)], nparts=32),
                        _mk(ps, 6 * 74, 64, [(74, 6), (1, 10)], nparts=32))

            # ---------------- child phase ----------------
            sgA = pool.tile([128, Q * 64], F32, name=f"sgA{kl}", tag="sgA")
            omA = pool.tile([128, Q * 64], F32, name=f"omA{kl}", tag="omA")
            muA = pool.tile([128, Q * K], F32, name=f"muA{kl}", tag="muA")
            scc = pool.tile([128, SCE], F32, name=f"scc{kl}", tag="scc")
            rcc = pool.tile([128, Q], F32, name=f"rcc{kl}", tag="rcc")
            sccP = pool.tile([128, 16 * Q], F32, name=f"sccP{kl}", tag="sccP")
            rccP = pool.tile([128, 2 * Q], F32, name=f"rccP{kl}", tag="rccP")
            ppoa = pool.tile([128, Q], F32, name=f"ppoa{kl}", tag="ppoa")
            ppsa = pool.tile([128, 2 * Q], F32, name=f"ppsa{kl}", tag="ppsa")

            for r in range(4):
                for t, nm in ((sgA, nmsa), (omA, nmoa)):
                    nc.sync.dma_start(
                        out=_mk(t, 0, 32 * r * Q * 64, [(64, Q), (1, 64)],
                                nparts=32, pstep=Q * 64),
                        in_=_mk(dr[nm], 0, r * 64, [(256, Q), (1, 64)],
                                nparts=32, pstep=FIB * 64))
                nc.sync.dma_start(
                    out=_mk(muA, 0, 32 * r * Q * K, [(K, Q), (1, K)],
                            nparts=32, pstep=Q * K),
                    in_=_mk(dr[nmua], 0, r * K, [(4 * K, Q), (1, K)],
                            nparts=32, pstep=FIB * K))

            nc.scalar.copy(_mk(augC, CE, 0, [(K * WC, Q), (WC, K), (1, K)]),
                           _mk(omA, Q * 64, 0, [(64, Q), (8, K), (1, K)]))
            nc.scalar.copy(_mk(augC, CE, 16, [(K * WC, Q), (WC, K)]),
                           _mk(muA, Q * K, 0, [(K, Q), (1, K)]))
            for i in range(K):
                nc.vector.tensor_tensor(
                    _mk(scc, SCE, 0, [(K, Q), (1, K)]),
                    _mk(muA, Q * K, 0, [(K, Q), (1, K)]),
                    _mk(muA, Q * K, i, [(K, Q), (0, K)]),
                    ALU.mult)
                nc.vector.tensor_tensor(
                    _mk(augC, CE, i * WC + 8, [(K * WC, Q), (1, K)]),
                    _mk(scc, SCE, 0, [(K, Q), (1, K)]),
                    _mk(sgA, Q * 64, i * 8, [(64, Q), (1, K)]),
                    ALU.add)

            _gj(nc, augC, CE, WC, Q, scc, SCE, rcc, Q, ppoa, nc.vector)
            _gj_logdet(nc, sgA, Q * 64, Q, sccP, 16 * Q, rccP, 2 * Q, ppsa,
                       nc.gpsimd)

            nc.vector.tensor_tensor(
                _mk(scc, SCE, 0, [(64, Q), (8, K), (1, K)]),
                _mk(augC, CE, 8, [(K * WC, Q), (WC, K), (1, K)]),
                _mk(augC, CE, 17, [(K * WC, Q), (WC, K), (1, K)]),
                ALU.mult)
            nc.vector.tensor_reduce(
                _mk(scc, SCE, Q * 64, [(1, Q)]),
                _mk(scc, SCE, 0, [(64, Q), (8, K), (1, K)]),
                mybir.AxisListType.XY, ALU.add)
            nc.vector.tensor_tensor(
                _mk(scc, SCE, Q * 65, [(K, Q), (1, K)]),
                _mk(augC, CE, 16, [(K * WC, Q), (WC, K)]),
                _mk(aux, Q * 10, 0, [(10, Q), (1, K)]),
                ALU.mult)
            nc.vector.tensor_reduce(
                _mk(scc, SCE, Q * 73, [(1, Q)]),
                _mk(scc, SCE, Q * 65, [(K, Q), (1, K)]),
                mybir.AxisListType.X, ALU.add)
            nc.scalar.activation(_mk(scc, SCE, Q * 74, [(1, Q)]),
                                 _mk(ppoa, Q, 0, [(1, Q)]), ACTF.Square)
            nc.scalar.activation(_mk(scc, SCE, Q * 74, [(1, Q)]),
                                 _mk(scc, SCE, Q * 74, [(1, Q)]), ACTF.Ln)
            nc.scalar.activation(_mk(scc, SCE, Q * 75, [(1, Q)]),
                                 _mk(ppsa, 2 * Q, 0, [(1, Q)]), ACTF.Ln)
            nc.vector.tensor_tensor(_mk(scc, SCE, Q * 74, [(1, Q)]),
                                    _mk(scc, SCE, Q * 74, [(1, Q)]),
                                    _mk(scc, SCE, Q * 75, [(1, Q)]),
                                    ALU.subtract)
            nc.vector.scalar_tensor_tensor(
                _mk(scc, SCE, Q * 76, [(1, Q)]),
                _mk(scc, SCE, Q * 73, [(1, Q)]), -2.0,
                _mk(scc, SCE, Q * 64, [(1, Q)]), ALU.mult, ALU.add)
            nc.vector.tensor_tensor(
                _mk(scc, SCE, Q * 76, [(1, Q)]),
                _mk(scc, SCE, Q * 76, [(1, Q)]),
                _mk(aux, Q * 10, 8, [(10, Q)]), ALU.add)
            nc.vector.tensor_tensor(
                _mk(scc, SCE, Q * 77, [(1, Q)]),
                _mk(scc, SCE, Q * 74, [(1, Q)]),
                _mk(aux, Q * 10, 9, [(10, Q)]), ALU.mult)
            nc.vector.tensor_tensor(
                _mk(scc, SCE, Q * 76, [(1, Q)]),
                _mk(scc, SCE, Q * 76, [(1, Q)]),
                _mk(scc, SCE, Q * 77, [(1, Q)]), ALU.add)
            nc.vector.tensor_reduce(
                _mk(scc, SCE, Q * 78, [(1, 1)]),
                _mk(scc, SCE, Q * 76, [(1, Q)]),
                mybir.AxisListType.X, ALU.add)
            nc.scalar.activation(_mk(outsb, 2, kl, [(1, 1)]),
                                 _mk(scc, SCE, Q * 78, [(1, 1)]),
                                 ACTF.Copy, scale=0.5)

        nc.sync.dma_start(out=out[:, :], in_=outsb[:, :])
    nc.finalize()
    return nc


def _build():
    if 'nc' not in _nc_cache:
        _nc_cache['nc'] = build_bass()
    return _nc_cache['nc']


def kernel(**inputs):
    nc = _build()
    ins = {k: np.ascontiguousarray(np.asarray(v, dtype=np.float32))
           for k, v in inputs.items()}
    in_maps = []
    for c in range(8):
        sl = slice(c * GAc, (c + 1) * GAc)
        m = {}
        for nm in ('mu_p', 'sigma_p', 'omega_c', 'mu_q', 'sigma_q', 'omega_par',
                   'mu_r', 'sigma_r', 'omega_model_c', 'mu_s', 'sigma_s',
                   'omega_model_p'):
            m[nm] = np.ascontiguousarray(ins[nm][:, sl])
        m['W'] = np.ascontiguousarray(ins['W'][:, :, sl])
        in_maps.append(m)
    res = run_bass_kernel_spmd(nc, in_maps, core_ids=list(range(8)))
    parts = np.stack([r['out'] for r in res.results])   # [8, 128, 2]
    belief = np.float32(parts[:, :, 0].astype(np.float64).sum())
    model = np.float32(parts[:, :, 1].astype(np.float64).sum())
    total = np.float32(np.float64(belief) + np.float64(model))
    return total, belief, model


if __name__ == '__main__':
    build_bass()
    print("built OK")
